# revision 1
# baseline (speedup 1.0000x reference)
"""L2SquaredConv2d (1x1 conv) on 8 TRN2 NeuronCores.

out[b,p,h,w] = relu( sum_c x[b,c,h,w]^2  - 2*sum_c x[b,c,h,w]*w[p,c] + sum_c w[p,c]^2 )

Strategy: data-parallel over batch (B=32 -> 4 images/core). Per core one big
bf16 matmul [P=2000, C=512] x [C, N=3136] done as 16 p-chunks x 4 images x
2 half-image n-tiles x 4 k-chunks, PSUM-accumulated in f32 ([128,784] 2-bank
PSUM tiles).

The i2[n] = sum_c x^2 term is computed by a matmul with an all-ones [128,128]
stationary operand: every output partition receives the same column sum, so the
reduction and the partition-broadcast happen in one PE pass. w2[p] is computed
by ScalarE Square activation with accum_out (fused sum over free dim) on the
[P, C]-layout copy of the weights. Eviction is fused and batched per p-chunk:
  VectorE: v[:, img] = -2*psum + i2r[:, img]   (scalar_tensor_tensor) x4
  ScalarE: o = relu(v + w2[p])                 (one [128,3136] activation)
  4 merged output DMAs (bf16), one per image.
"""

import numpy as np
import ml_dtypes

import concourse.bacc as bacc
import concourse.bass as bass
import concourse.mybir as mybir
import concourse.tile as tile
from concourse import bass_utils

B, C, H, W = 32, 512, 28, 28
P = 2000
NCORES = 8
BL = B // NCORES          # 4 images per core
HW = H * W                # 784
N = BL * HW               # 3136 pixels per core
KC = C // 128             # 4 contraction chunks
TN = 392                  # matmul moving-dim tile (half an image)
PC = (P + 127) // 128     # 16 p-chunks (last one is 80 rows)
P_PAD = PC * 128

BF16 = mybir.dt.bfloat16
F32 = mybir.dt.float32
NPBF16 = ml_dtypes.bfloat16

_CACHE = {}


def _build():
    nc = bacc.Bacc(
        "TRN2", target_bir_lowering=False, debug=False, num_devices=NCORES
    )
    xT_d = nc.dram_tensor("xT", [KC, 128, N], BF16, kind="ExternalInput")
    wT_d = nc.dram_tensor("wT", [KC, 128, P], BF16, kind="ExternalInput")
    wpc_d = nc.dram_tensor("w_pc", [PC, 128, C], BF16, kind="ExternalInput")
    out_d = nc.dram_tensor("out", [BL, P, HW], BF16, kind="ExternalOutput")
    ones_d = nc.inline_tensor(np.ones((128, 128), dtype=NPBF16), "ones_mat")

    RELU = mybir.ActivationFunctionType.Relu
    SQUARE = mybir.ActivationFunctionType.Square

    with tile.TileContext(nc) as tc:
        with (
            tc.tile_pool(name="resident", bufs=1) as rpool,
            tc.tile_pool(name="x2p", bufs=2) as x2_pool,
            tc.tile_pool(name="wpc", bufs=4) as wpc_pool,
            tc.tile_pool(name="sq", bufs=2) as sq_pool,
            tc.tile_pool(name="v", bufs=3) as v_pool,
            tc.tile_pool(name="o", bufs=3) as o_pool,
            tc.tile_pool(name="pm", bufs=3, space=bass.MemorySpace.PSUM) as pm_pool,
            tc.tile_pool(name="pi", bufs=1, space=bass.MemorySpace.PSUM) as pi_pool,
        ):
            # ---- resident tiles ----
            x_sb = [rpool.tile([128, N], BF16, tag=f"x{k}", name=f"x{k}") for k in range(KC)]
            wt_sb = [rpool.tile([128, P], BF16, tag=f"w{k}", name=f"w{k}") for k in range(KC)]
            ones_sb = rpool.tile([128, 128], BF16, tag="ones")
            w2col = rpool.tile([128, PC], F32, tag="w2col")
            i2r = rpool.tile([128, N], F32, tag="i2r")

            # ---- input DMAs, ordered so compute can start early ----
            # image 0 of x (everything for the first i2 + first matmuls)
            nc.sync.dma_start(ones_sb[:], ones_d[:])
            for k in range(KC):
                nc.sync.dma_start(x_sb[k][:, 0:HW], xT_d[k, :, 0:HW])
            # first columns of wT (p-chunks 0..3)
            for k in range(KC):
                nc.sync.dma_start(wt_sb[k][:, 0:512], wT_d[k, :, 0:512])
            # first p-chunks of w_pc (feeds w2col for the first evictions)
            wpc_t = []
            for pc_i in range(PC):
                t = wpc_pool.tile([128, C], BF16, name=f"wpc{pc_i}")
                wpc_t.append(t)
                if pc_i < 4:
                    nc.sync.dma_start(t[:], wpc_d[pc_i])
            # rest of x
            for k in range(KC):
                nc.sync.dma_start(x_sb[k][:, HW:N], xT_d[k, :, HW:N])
            # rest of w_pc
            for pc_i in range(4, PC):
                nc.sync.dma_start(wpc_t[pc_i][:], wpc_d[pc_i])
            # rest of wT
            for k in range(KC):
                nc.sync.dma_start(wt_sb[k][:, 512:P], wT_d[k, :, 512:P])

            # ---- w2[p] = sum_c w[p,c]^2 (ScalarE square + accumulate) ----
            for pc_i in range(PC):
                sq_t = sq_pool.tile([128, C], BF16)
                nc.scalar.activation(
                    sq_t[:], wpc_t[pc_i][:], SQUARE,
                    accum_out=w2col[:, pc_i:pc_i + 1],
                )

            # ---- i2 broadcast rows: ones.T @ x^2, one group per image ----
            for img in range(BL):
                isl = slice(img * HW, (img + 1) * HW)
                x2t = [x2_pool.tile([128, HW], BF16, tag=f"x2_{k}", name=f"x2_{k}")
                       for k in range(KC)]
                for k in range(KC):
                    nc.vector.tensor_mul(x2t[k][:], x_sb[k][:, isl],
                                         x_sb[k][:, isl])
                pi = pi_pool.tile([128, HW], F32)
                for off, nn in ((0, 512), (512, 272)):
                    hsl = slice(off, off + nn)
                    for k in range(KC):
                        nc.tensor.matmul(
                            pi[:, hsl], ones_sb[:], x2t[k][:, hsl],
                            start=(k == 0), stop=(k == KC - 1),
                        )
                nc.vector.tensor_copy(i2r[:, isl], pi[:])

            # ---- main loop: p-chunk outer, image inner ----
            for p_i in range(PC):
                M = min(128, P - p_i * 128)
                psl = slice(p_i * 128, p_i * 128 + M)
                v = v_pool.tile([128, N], F32)
                for img in range(BL):
                    isl = slice(img * HW, (img + 1) * HW)
                    ps = pm_pool.tile([128, HW], F32)
                    for off, nn in ((0, 512), (512, 272)):
                        for k in range(KC):
                            nc.tensor.matmul(
                                ps[:M, off:off + nn],
                                wt_sb[k][:, psl],
                                x_sb[k][:, img * HW + off:img * HW + off + nn],
                                start=(k == 0), stop=(k == KC - 1),
                            )
                    nc.vector.scalar_tensor_tensor(
                        v[:M, isl], ps[:M, :], -2.0, i2r[:M, isl],
                        op0=mybir.AluOpType.mult, op1=mybir.AluOpType.add,
                    )
                o = o_pool.tile([128, N], BF16)
                nc.scalar.activation(
                    o[:M], v[:M], RELU, bias=w2col[:M, p_i:p_i + 1], scale=1.0,
                )
                for img in range(BL):
                    nc.sync.dma_start(
                        out_d[img, psl, :], o[:M, img * HW:(img + 1) * HW]
                    )

    nc.compile()
    return nc


def _get_nc():
    if "nc" not in _CACHE:
        _CACHE["nc"] = _build()
    return _CACHE["nc"]


def _make_in_maps(input, weights):
    x = np.asarray(input, dtype=np.float32)
    w = np.asarray(weights, dtype=np.float32).reshape(P, C)

    wT = np.ascontiguousarray(w.T).astype(NPBF16).reshape(KC, 128, P)
    w_pad = np.zeros((P_PAD, C), np.float32)
    w_pad[:P] = w
    w_pc = w_pad.astype(NPBF16).reshape(PC, 128, C)

    in_maps = []
    for c in range(NCORES):
        sh = x[c * BL:(c + 1) * BL]                      # [4, 512, 28, 28]
        xT = np.ascontiguousarray(
            sh.transpose(1, 0, 2, 3).reshape(C, N)
        ).astype(NPBF16).reshape(KC, 128, N)
        in_maps.append({"xT": xT, "wT": wT, "w_pc": w_pc})
    return in_maps


def run(input, weights, trace=False):
    """Returns (output [32,2000,28,28] f32, BassKernelResults)."""
    nc = _get_nc()
    in_maps = _make_in_maps(input, weights)
    res = bass_utils.run_bass_kernel_spmd(
        nc, in_maps, core_ids=list(range(NCORES)), trace=trace
    )
    outs = [res.results[c]["out"] for c in range(NCORES)]   # [4, 2000, 784] bf16
    out = (
        np.concatenate(outs, axis=0).astype(np.float32).reshape(B, P, H, W)
    )
    return out, res


def kernel(input, weights):
    out, _ = run(input, weights, trace=False)
    return out

